# revision 13
# baseline (speedup 1.0000x reference)
"""Trainium2 Bass kernel for nn_DSC_AttentionBaseline (sparse_attention).

Sharding: the 16 (batch, head) units run 2-per-core on 8 cores (cores 0-3 ->
image 0, cores 4-7 -> image 1). Each core computes q/k/v, per-head attention,
the AiA conv branch (c0 + two snake-deformable convs + c1), the second
attention and attn@v for its two heads; the final output projection (mixes
all heads of an image) uses an AllGather across each 4-core group. The host
only reshapes/shards inputs and concatenates per-core output slices.

Device-side techniques:
  * convs = shifted matmuls over a 34x34 zero-padded layout with slack, so
    every operand is a single-stride AP (garbage lands on pad cells, never
    read back);
  * snake sampling is exact: offsets along one axis are integers, so the
    bilinear sample is a tent-weighted contraction over 32 source rows/cols,
    executed on the tensor engine with diagonal tile_position packing;
  * softmax is folded into attn@v by appending a ones column to v^T (yields
    unnormalized rows and the row sum in one PSUM accumulation).
"""
import os
import sys

sys.path.insert(0, "/opt/trn_rl_repo")
import numpy as np
import concourse.bass as bass
import concourse.bacc as bacc
import concourse.mybir as mybir
import concourse.tile as tile
from concourse.bass_utils import run_bass_kernel_spmd

F32 = mybir.dt.float32
AF = mybir.ActivationFunctionType
ALU = mybir.AluOpType

N_CORES = 8
NC, H, W = 256, 32, 32
M = H * W
NH, HC, K9, CENTER = 8, 32, 9, 4
SCALE = HC ** -0.5
EPS = 1e-5

PW = 34
PL = PW * PW
SL = 36
FT = SL + PL + SL          # 1228
RG = [(0, 12), (12, 12), (24, 10)]

DEBUG = os.environ.get("BK_DEBUG", "") == "1"
_CACHE = {}


def _interior(rg_idx):
    r0, nr = RG[rg_idx]
    y0 = max(r0, 1) - 1
    y1 = min(r0 + nr, 33) - 1
    return y0, y1 - y0


def build_program():
    if "nc" in _CACHE:
        return _CACHE["nc"]
    nc = bacc.Bacc("TRN2", target_bir_lowering=False, debug=False,
                   num_devices=N_CORES)

    def din(name, shape):
        return nc.dram_tensor(name, shape, F32, kind="ExternalInput").ap()

    def dout(name, shape):
        return nc.dram_tensor(name, shape, F32, kind="ExternalOutput").ap()

    io = dict(
        xf=din("xf", [128, 2048]),
        wqT=din("wqT", [128, 128]), wkT=din("wkT", [128, 128]),
        wvT=din("wvT", [128, 128]),
        bq2=din("bq2", [64, 1]), bk2=din("bk2", [64, 1]),
        bv2=din("bv2", [128, 64]),
        p1wT=din("p1wT", [128, 8 * 256]), p1b=din("p1b", [128, 2]),
        p2wT=din("p2wT", [128, 2 * 1024]),
        p2l=din("p2l", [2, 1024]), p2r=din("p2r", [2, 1024]),
        c0wT=din("c0wT", [128, 9 * 2 * 256]),
        offwT=din("offwT", [128, 9 * 2 * 41]),
        dscwT=din("dscwT", [128, 2 * 9 * 2 * 256]),
        c1wT=din("c1wT", [128, 9 * 6 * 256]),
        c0b=din("c0b", [128, 2]), offb=din("offb", [41, 1]),
        dscb=din("dscb", [128, 4]), c1b=din("c1b", [128, 2]),
        c0g=din("c0g", [128, 2]), c0be=din("c0be", [128, 2]),
        offg=din("offg", [41, 1]), offbe=din("offbe", [41, 1]),
        dscg=din("dscg", [128, 4]), dscbe=din("dscbe", [128, 4]),
        c1g=din("c1g", [128, 2]), c1be=din("c1be", [128, 2]),
        woT=din("woT", [128, 2 * 64]), bo64=din("bo64", [64, 1]),
        ident=din("ident", [128, 128]), sel4=din("sel4", [4, 128]),
        tri=din("tri", [10, 9]), niota=din("niota", [128, 1]),
        g4=din("g4", [128, 32]), g4t=din("g4t", [32, 128]),
        gp2=din("gp2", [41, 18]), gp2t=din("gp2t", [18, 41]),
        gridy=din("gridy", [1, FT]), gridx=din("gridx", [1, FT]),
        ypart=dout("ypart", [64, 1024]),
    )
    if DEBUG:
        for nm, sh in [("dbg_q", [64, 1024]), ("dbg_attn1", [128, 1024]),
                       ("dbg_fm", [128, 2 * FT]), ("dbg_offs", [41, 1024]),
                       ("dbg_tanh", [18, FT]), ("dbg_yc", [9, FT]),
                       ("dbg_T4", [128, 256]), ("dbg_P4", [128, 256]),
                       ("dbg_dsc", [128, 1024]), ("dbg_fmcat", [128, 6 * FT]),
                       ("dbg_fmc", [128, 2 * 1024]), ("dbg_E", [128, 1024]),
                       ("dbg_gsrc", [64, 1024])]:
            io[nm] = dout(nm, sh)

    with tile.TileContext(nc) as tc:
        _emit(nc, tc, io)
    nc.compile()
    _CACHE["nc"] = nc
    return nc


def _emit(nc, tc, io):
    from contextlib import ExitStack

    with ExitStack() as top:
        wp = top.enter_context(tc.tile_pool(name="wpers", bufs=1))
        act = top.enter_context(tc.tile_pool(name="acts", bufs=1))
        dramp = top.enter_context(tc.tile_pool(name="dramp", bufs=1,
                                               space="DRAM"))

        wearly_ctx = tc.tile_pool(name="wearly", bufs=1)
        we = wearly_ctx.__enter__()

        def load(name, shape, pool=None):
            t = (pool or wp).tile(shape, F32, tag=name)
            nc.sync.dma_start(t[:], io[name][:])
            return t

        xft = load("xf", [128, 2048], we)
        wqTt = load("wqT", [128, 128], we); wkTt = load("wkT", [128, 128], we)
        wvTt = load("wvT", [128, 128], we)
        bq2t = load("bq2", [64, 1], we); bk2t = load("bk2", [64, 1], we)
        bv2t = load("bv2", [128, 64], we)
        p1wTt = load("p1wT", [128, 8 * 256], we)
        p1bt = load("p1b", [128, 2], we)
        p2lt = load("p2l", [2, 1024]); p2rt = load("p2r", [2, 1024])
        woTt = load("woT", [128, 2 * 64]); bo64t = load("bo64", [64, 1])
        identt = load("ident", [128, 128]); sel4t = load("sel4", [4, 128])
        trit = load("tri", [10, 9]); niotat = load("niota", [128, 1])
        g4t_ = load("g4", [128, 32]); g4tT = load("g4t", [32, 128])
        gp2_ = load("gp2", [41, 18]); gp2T = load("gp2t", [18, 41])
        c0bt = load("c0b", [128, 2]); offbt = load("offb", [41, 1])
        dscbt = load("dscb", [128, 4]); c1bt = load("c1b", [128, 2])
        c0gt = load("c0g", [128, 2]); c0bet = load("c0be", [128, 2])
        offgt = load("offg", [41, 1]); offbet = load("offbe", [41, 1])
        dscgt = load("dscg", [128, 4]); dscbet = load("dscbe", [128, 4])
        c1gt = load("c1g", [128, 2]); c1bet = load("c1be", [128, 2])
        ones132 = wp.tile([1, 32], F32, name="ones132", tag="ones132")
        nc.gpsimd.memset(ones132[:], 1.0)
        epscol = wp.tile([128, 1], F32, name="epscol", tag="epscol")
        nc.gpsimd.memset(epscol[:], EPS)

        fm = [act.tile([128, 2 * FT], F32, name=f"fm{u}", tag=f"fm{u}") for u in range(2)]
        fmcat = act.tile([128, 6 * FT], F32, name="fmcat", tag="fmcat")
        fmc = [act.tile([128, 2 * 1024], F32, name=f"fmc{u}", tag=f"fmc{u}") for u in range(2)]
        vte = act.tile([128, 8 * 66], F32, name="vte", tag="vte")
        q2 = act.tile([64, 1024], F32, name="q2", tag="q2")
        k2 = act.tile([64, 1024], F32, name="k2", tag="k2")
        gsrc = act.tile([64, 1024], F32, name="gsrc", tag="gsrc")
        scr = [act.tile([128, 1024], F32, name=f"scr{c}", tag=f"scr{c}") for c in range(2)]
        sqs = act.tile([128, 1024], F32, name="sqs", tag="sqs")
        offsc = act.tile([41, 1024], F32, name="offsc", tag="offsc")
        stats = act.tile([128, 8], F32, name="stats", tag="stats")
        scol2 = act.tile([128, 2], F32, name="scol2", tag="scol2")
        qcol2 = act.tile([128, 2], F32, name="qcol2", tag="qcol2")
        dssum = [act.tile([128, 8], F32, name=f"dssum{c}", tag=f"dssum{c}") for c in range(2)]
        ofstats = act.tile([41, 8], F32, name="ofstats", tag="ofstats")
        ofred = act.tile([41, 2], F32, name="ofred", tag="ofred")
        cumin = [act.tile([10, FT], F32, name=f"cumin{br}", tag=f"cumin{br}") for br in range(2)]
        ycs = [act.tile([9, FT], F32, name=f"yc{br}", tag=f"yc{br}") for br in range(2)]
        yctx = act.tile([9, 1024], F32, name="yctx", tag="yctx")

        def img(t, cc):
            return t[:, FT * cc + SL: FT * cc + SL + PL].rearrange(
                "p (y x) -> p y x", x=PW)

        for u in range(2):
            nc.gpsimd.memset(fm[u][:], 0.0)
        nc.gpsimd.memset(fmcat[:], 0.0)
        for br in range(2):
            nc.gpsimd.memset(cumin[br][:], 0.0)
        nc.sync.dma_start(cumin[0][9:10, :], io["gridy"][:])
        nc.sync.dma_start(cumin[1][9:10, :], io["gridx"][:])

        # ================= P1: q/k/v =================
        with tc.tile_pool(name="ps1", bufs=2, space="PSUM") as pq:
            for (dst, wT, bcol) in ((q2, wqTt, bq2t), (k2, wkTt, bk2t)):
                for mc in range(8):
                    ps = pq.tile([64, 128], F32, name="pqk", tag="pqk")
                    for cc in range(2):
                        nc.tensor.matmul(
                            ps[:], wT[:, 64 * cc:64 * (cc + 1)],
                            xft[:, 1024 * cc + 128 * mc:
                                1024 * cc + 128 * (mc + 1)],
                            start=(cc == 0), stop=(cc == 1))
                    nc.scalar.activation(dst[:, 128 * mc:128 * (mc + 1)],
                                         ps[:], AF.Identity, bias=bcol[:])
            for mc in range(8):
                ps = pq.tile([128, 64], F32, name="pv", tag="pv")
                for cc in range(2):
                    nc.tensor.matmul(
                        ps[:], xft[:, 1024 * cc + 128 * mc:
                                   1024 * cc + 128 * (mc + 1)],
                        wvTt[:, 64 * cc:64 * (cc + 1)],
                        start=(cc == 0), stop=(cc == 1))
                vsl = vte[:, 66 * mc:66 * (mc + 1)]
                nc.vector.tensor_tensor(ps[:], ps[:], bv2t[:], ALU.add)
                nc.vector.tensor_copy(vsl[:, 0:32], ps[:, 0:32])
                nc.vector.tensor_copy(vsl[:, 33:65], ps[:, 32:64])
                nc.gpsimd.memset(vsl[:, 32:33], 1.0)
                nc.gpsimd.memset(vsl[:, 65:66], 1.0)
        if DEBUG:
            nc.sync.dma_start(io["dbg_q"][:], q2[:])

        # ================= P2: attn1 + p1 -> fm =================
        with tc.tile_pool(name="ps2", bufs=1, space="PSUM") as pa, \
             tc.tile_pool(name="ps2b", bufs=2, space="PSUM") as pab, \
             tc.tile_pool(name="sb2", bufs=2) as sa:
            for u in range(2):
                p1ps = [pa.tile([128, 1024], F32, name=f"p1ps{c}", tag=f"p1ps{c}")
                        for c in range(2)]
                for mc in range(8):
                    aps = pab.tile([128, 1024], F32, name="a1ps", tag="a1ps")
                    for ns in range(2):
                        nc.tensor.matmul(
                            aps[:, 512 * ns:512 * (ns + 1)],
                            q2[32 * u:32 * (u + 1), 128 * mc:128 * (mc + 1)],
                            k2[32 * u:32 * (u + 1), 512 * ns:512 * (ns + 1)],
                            start=True, stop=True)
                    a1 = sa.tile([128, 1024], F32, name="a1sb", tag="a1sb")
                    if mc % 2 == 0:
                        nc.vector.tensor_copy(a1[:], aps[:])
                    else:
                        nc.scalar.copy(a1[:], aps[:])
                    if DEBUG and u == 0 and mc == 0:
                        nc.sync.dma_start(io["dbg_attn1"][:], a1[:])
                    for cc in range(2):
                        for ns in range(2):
                            nc.tensor.matmul(
                                p1ps[cc][:, 512 * ns:512 * (ns + 1)],
                                p1wTt[:, 256 * mc + 128 * cc:
                                      256 * mc + 128 * (cc + 1)],
                                a1[:, 512 * ns:512 * (ns + 1)],
                                start=(mc == 0), stop=(mc == 7))
                for cc in range(2):
                    nc.scalar.activation(
                        img(fm[u], cc)[:, 1:33, 1:33],
                        p1ps[cc][:].rearrange("p (y x) -> p y x", x=32),
                        AF.Identity, bias=p1bt[:, cc:cc + 1])
        if DEBUG:
            nc.sync.dma_start(io["dbg_fm"][:], fm[0][:])
        wearly_ctx.__exit__(None, None, None)

        # ============ helper: group-norm scale/bias ============
        def gn_scale_bias(psg, nchan, scol, qcol, n_elems, gind, gindT, gam,
                          bet, ngroups, tg):
            sp = act.tile([nchan, 2], F32, name=f"gnsp{tg}", tag=f"gnsp{tg}")
            nc.vector.tensor_copy(sp[:, 0:1], scol)
            nc.vector.tensor_copy(sp[:, 1:2], qcol)
            gps = psg.tile([ngroups, 2], F32, name=f"gnps{tg}", tag=f"gnps{tg}")
            nc.tensor.matmul(gps[:], gind[:], sp[:], start=True, stop=True)
            mv = act.tile([ngroups, 2], F32, name=f"gnmv{tg}", tag=f"gnmv{tg}")
            nc.vector.tensor_scalar_mul(mv[:], gps[:], 1.0 / n_elems)
            nv = act.tile([ngroups, 1], F32, name=f"gnnv{tg}", tag=f"gnnv{tg}")
            nc.vector.scalar_tensor_tensor(nv[:], mv[:, 0:1], mv[:, 0:1],
                                           mv[:, 1:2], ALU.mult, ALU.subtract)
            sd = act.tile([ngroups, 1], F32, name=f"gnsd{tg}", tag=f"gnsd{tg}")
            nc.scalar.activation(sd[:], nv[:], AF.Sqrt, bias=epscol[0:ngroups, :], scale=-1.0)
            rs = act.tile([ngroups, 1], F32, name=f"gnrs{tg}", tag=f"gnrs{tg}")
            nc.vector.reciprocal(rs[:], sd[:])
            mr = act.tile([ngroups, 2], F32, name=f"gnmr{tg}", tag=f"gnmr{tg}")
            nc.vector.tensor_copy(mr[:, 0:1], mv[:, 0:1])
            nc.vector.tensor_copy(mr[:, 1:2], rs[:])
            bc = psg.tile([nchan, 2], F32, name=f"gnbc{tg}", tag=f"gnbc{tg}")
            nc.tensor.matmul(bc[:], gindT[:], mr[:], start=True, stop=True)
            S = act.tile([nchan, 1], F32, name=f"gnS{tg}", tag=f"gnS{tg}")
            nc.vector.tensor_tensor(S[:], bc[:, 1:2], gam, ALU.mult)
            t_ = act.tile([nchan, 1], F32, name=f"gnt{tg}", tag=f"gnt{tg}")
            nc.vector.tensor_tensor(t_[:], bc[:, 0:1], S[:], ALU.mult)
            Bc = act.tile([nchan, 1], F32, name=f"gnB{tg}", tag=f"gnB{tg}")
            nc.vector.tensor_sub(Bc[:], bet, t_[:])
            return S, Bc

        # ============ P3..P6 per unit ============
        for u in range(2):
            fmu = fm[u]
            # ---- P3a: c0 conv (2 out chunks) ----
            with tc.tile_pool(name="ps3a", bufs=1, space="PSUM") as pc, \
                 tc.tile_pool(name="sbw3", bufs=3) as sbw:
                cps = [[pc.tile([128, RG[rg][1] * PW], F32, name=f"c0ps{rg}{oc}", tag=f"c0ps{rg}{oc}")
                        for oc in range(2)] for rg in range(3)]
                for si in range(9):
                    dy, dx = si // 3 - 1, si % 3 - 1
                    delta = PW * dy + dx
                    blk = sbw.tile([128, 512], F32, name="c0blk", tag="c0blk")
                    nc.sync.dma_start(blk[:], io["c0wT"][:, si * 512:
                                                         (si + 1) * 512])
                    for cc in range(2):
                        for rg in range(3):
                            r0, nr = RG[rg]
                            nfree = nr * PW
                            rhs = fmu[:, FT * cc + SL + r0 * PW + delta:
                                      FT * cc + SL + r0 * PW + delta + nfree]
                            for oc in range(2):
                                nc.tensor.matmul(
                                    cps[rg][oc][:],
                                    blk[:, 256 * cc + 128 * oc:
                                        256 * cc + 128 * (oc + 1)],
                                    rhs,
                                    start=(si == 0 and cc == 0),
                                    stop=(si == 8 and cc == 1))
                for rg in range(3):
                    r0, nr = RG[rg]
                    y0, ny = _interior(rg)
                    for oc in range(2):
                        src = cps[rg][oc][:].rearrange(
                            "p (y x) -> p y x", x=PW)[
                            :, y0 + 1 - r0:y0 + 1 - r0 + ny, 1:33]
                        dst = scr[oc][:].rearrange(
                            "p (y x) -> p y x", x=32)[:, y0:y0 + ny, :]
                        nc.scalar.activation(
                            dst, src, AF.Identity, bias=c0bt[:, oc:oc + 1],
                            accum_out=stats[:, 3 * oc + rg:3 * oc + rg + 1])
            # ---- P3b: offsets conv ----
            with tc.tile_pool(name="ps3b", bufs=1, space="PSUM") as pc, \
                 tc.tile_pool(name="sbw3b", bufs=3) as sbw:
                ops_ = [pc.tile([41, RG[rg][1] * PW], F32, name=f"ofps{rg}", tag=f"ofps{rg}")
                        for rg in range(3)]
                for si in range(9):
                    dy, dx = si // 3 - 1, si % 3 - 1
                    delta = PW * dy + dx
                    blk = sbw.tile([128, 82], F32, name="offblk", tag="offblk")
                    nc.sync.dma_start(blk[:], io["offwT"][:, si * 82:
                                                          (si + 1) * 82])
                    for cc in range(2):
                        for rg in range(3):
                            r0, nr = RG[rg]
                            nfree = nr * PW
                            rhs = fmu[:, FT * cc + SL + r0 * PW + delta:
                                      FT * cc + SL + r0 * PW + delta + nfree]
                            nc.tensor.matmul(
                                ops_[rg][:],
                                blk[:, 41 * cc:41 * (cc + 1)], rhs,
                                start=(si == 0 and cc == 0),
                                stop=(si == 8 and cc == 1))
                for rg in range(3):
                    r0, nr = RG[rg]
                    y0, ny = _interior(rg)
                    src = ops_[rg][:].rearrange("p (y x) -> p y x", x=PW)[
                        :, y0 + 1 - r0:y0 + 1 - r0 + ny, 1:33]
                    dst = offsc[:].rearrange("p (y x) -> p y x", x=32)[
                        :, y0:y0 + ny, :]
                    nc.scalar.activation(
                        dst, src, AF.Identity, bias=offbt[:],
                        accum_out=ofstats[:, rg:rg + 1])

            # ---- P3c: GN(c0)+ReLU -> fmcat, GN(off)+tanh -> cumins ----
            with tc.tile_pool(name="psgn", bufs=1, space="PSUM") as psg:
                for oc in range(2):
                    nc.vector.tensor_reduce(
                        scol2[:, oc:oc + 1], stats[:, 3 * oc:3 * oc + 3],
                        mybir.AxisListType.X, ALU.add)
                    nc.vector.scalar_tensor_tensor(
                        sqs[:], scr[oc][:], 1.0, scr[oc][:], ALU.mult,
                        ALU.mult, accum_out=qcol2[:, oc:oc + 1])
                    S, Bc = gn_scale_bias(
                        psg, 128, scol2[:, oc:oc + 1], qcol2[:, oc:oc + 1],
                        4 * 1024, g4t_[:], g4tT[:], c0gt[:, oc:oc + 1],
                        c0bet[:, oc:oc + 1], 32, "c")
                    nc.scalar.activation(
                        img(fmcat, oc)[:, 1:33, 1:33],
                        scr[oc][:].rearrange("p (y x) -> p y x", x=32),
                        AF.Relu, bias=Bc[:], scale=S[:])
                # offsets GN + tanh
                nc.vector.tensor_reduce(ofred[:, 0:1], ofstats[:, 0:3],
                                        mybir.AxisListType.X, ALU.add)
                nc.vector.scalar_tensor_tensor(
                    sqs[0:41, :], offsc[:], 1.0, offsc[:], ALU.mult, ALU.mult,
                    accum_out=ofred[:, 1:2])
                S, Bc = gn_scale_bias(
                    psg, 41, ofred[:, 0:1], ofred[:, 1:2], 2 * 1024,
                    gp2_[:], gp2T[:], offgt[:], offbet[:], 18, "o")
                if DEBUG and u == 0:
                    nc.sync.dma_start(io["dbg_offs"][:], offsc[:])
                for br, pbase in ((0, 0), (1, 32)):
                    nc.scalar.activation(
                        cumin[br][0:9, SL:SL + PL].rearrange(
                            "p (y x) -> p y x", x=PW)[:, 1:33, 1:33],
                        offsc[pbase:pbase + 9, :].rearrange(
                            "p (y x) -> p y x", x=32),
                        AF.Tanh, bias=Bc[pbase:pbase + 9, :],
                        scale=S[pbase:pbase + 9, :])
                if DEBUG and u == 0:
                    nc.sync.dma_start(io["dbg_tanh"][0:9, :], cumin[0][0:9, :])
                    nc.sync.dma_start(io["dbg_tanh"][9:18, :], cumin[1][0:9, :])
                # cumsum + grid via TRI matmul, then clip
                for br in range(2):
                    ycp = psg.tile([9, FT], F32, name="ycps", tag="ycps")
                    for s0, n0 in ((0, 512), (512, 512), (1024, FT - 1024)):
                        nc.tensor.matmul(ycp[:, s0:s0 + n0], trit[:],
                                         cumin[br][:, s0:s0 + n0],
                                         start=True, stop=True)
                    nc.vector.tensor_scalar(ycs[br][:], ycp[:], 31.0, 0.0,
                                            ALU.min, ALU.max)
                if DEBUG and u == 0:
                    nc.sync.dma_start(io["dbg_yc"][:], ycs[0][:])
                # yctx: (w,h)-transposed interior of yc_cx
                nc.vector.tensor_copy(
                    yctx[:].rearrange("p (w h) -> p w h", h=32),
                    ycs[0][:, SL:SL + PL].rearrange(
                        "p (y x) -> p y x", x=PW)[:, 1:33, 1:33]
                    .transpose([0, 2, 1]))

            # ---- P4/P5 per branch: tents, P tiles, sampling, GN ----
            for br in range(2):
                with tc.tile_pool(name="sbT", bufs=2) as sbT:
                    Ts = []
                    psTc = tc.tile_pool(name="psT", bufs=2, space="PSUM")
                    psT = psTc.__enter__()
                    for k in range(9):
                        ycW = sbT.tile([4, 256], F32, name="ycW", tag="ycW")
                        if br == 0:
                            v4 = yctx[:].rearrange(
                                "p (wg wl h) -> p wl wg h", wl=4, h=32)
                            for wl in range(4):
                                nc.sync.dma_start(
                                    ycW[wl:wl + 1, :].rearrange(
                                        "p (wg h) -> p wg h", h=32),
                                    v4[k:k + 1, wl])
                        else:
                            v4 = ycs[1][:, SL:SL + PL].rearrange(
                                "p (y x) -> p y x", x=PW)[:, 1:33, 1:33] \
                                .rearrange("p (hg hl) w -> p hl hg w", hl=4)
                            for hl in range(4):
                                nc.sync.dma_start(
                                    ycW[hl:hl + 1, :].rearrange(
                                        "p (hg w) -> p hg w", w=32),
                                    v4[k:k + 1, hl])
                        ycr = psT.tile([128, 256], F32, name="ycr", tag="ycr")
                        nc.tensor.matmul(ycr[:], sel4t[:], ycW[:],
                                         start=True, stop=True)
                        ut = sbT.tile([128, 256], F32, name="ut", tag="ut")
                        nc.scalar.activation(ut[:], ycr[:], AF.Abs,
                                             bias=niotat[:])
                        Tk = sbT.tile([128, 256], F32, name=f"T{k}", tag=f"T{k}", bufs=1)
                        nc.scalar.activation(Tk[:], ut[:], AF.Relu,
                                             bias=1.0, scale=-1.0)
                        Ts.append(Tk)
                    if DEBUG and u == 0 and br == 0:
                        nc.sync.dma_start(io["dbg_T4"][:], Ts[4][:])
                    psTc.__exit__(None, None, None)

                    with tc.tile_pool(name="psP", bufs=2, space="PSUM") as psP, \
                         tc.tile_pool(name="sbP", bufs=2) as sbP, \
                         tc.tile_pool(name="sbDW", bufs=1) as sbDW, \
                         tc.tile_pool(name="sb5", bufs=2) as sb5:
                        dblk = []
                        for k in range(9):
                            t = sbDW.tile([128, 512], F32, name=f"dwblk{k}", tag=f"dwblk{k}")
                            base = ((br * 9 + k) * 2) * 256
                            nc.sync.dma_start(
                                t[:], io["dscwT"][:, base:base + 512])
                            dblk.append(t)
                        for g in range(8):
                            ds = psP.tile([128, 256], F32, name="ds", tag="ds")
                            for k in range(9):
                                pp = psP.tile([128, 256], F32, name="pp",
                                              tag="pp")
                                for cc in range(2):
                                    for wl in range(4):
                                        if br == 0:
                                            xs = min(max(4 * g + wl + k - 4,
                                                         0), 31)
                                            lhsT = img(fmu, cc)[:, 1:33,
                                                                1 + xs]
                                        else:
                                            ys = min(max(4 * g + wl + k - 4,
                                                         0), 31)
                                            lhsT = img(fmu, cc)[:, 1 + ys,
                                                                1:33]
                                        nc.tensor.matmul(
                                            pp[32 * wl:32 * (wl + 1), :],
                                            lhsT,
                                            dblk[k][:, 256 * cc:
                                                    256 * (cc + 1)],
                                            start=(cc == 0), stop=(cc == 1),
                                            tile_position=(0, 32 * wl))
                                Pk = sbP.tile([128, 256], F32, name="Pk",
                                              tag="Pk", bufs=3)
                                if k % 2 == 0:
                                    nc.vector.tensor_copy(Pk[:], pp[:])
                                else:
                                    nc.scalar.copy(Pk[:], pp[:])
                                if DEBUG and u == 0 and br == 0 and g == 0 \
                                        and k == 4:
                                    nc.sync.dma_start(io["dbg_P4"][:], Pk[:])
                                for wl in range(4):
                                    nc.tensor.matmul(
                                        ds[32 * wl:32 * (wl + 1), :],
                                        Ts[k][32 * wl:32 * (wl + 1),
                                              32 * g:32 * (g + 1)],
                                        Pk[32 * wl:32 * (wl + 1), :],
                                        start=(k == 0), stop=(k == 8),
                                        tile_position=(32 * wl, 32 * wl))
                            dsb = sb5.tile([128, 256], F32, name="dsb", tag="dsb")
                            nc.vector.tensor_copy(dsb[:], ds[:])
                            for occ in range(2):
                                tp = psP.tile([128, 128], F32, name="tp", tag="tp")
                                nc.tensor.transpose(
                                    tp[:], dsb[:, 128 * occ:128 * (occ + 1)],
                                    identt[:])
                                tpv = tp[:].rearrange(
                                    "p (wl h) -> p wl h", h=32)
                                if br == 0:
                                    dst = scr[occ][:].rearrange(
                                        "p (h w) -> p h w", w=32)[
                                        :, :, 4 * g:4 * (g + 1)] \
                                        .transpose([0, 2, 1])
                                else:
                                    dst = scr[occ][:].rearrange(
                                        "p (h w) -> p h w", w=32)[
                                        :, 4 * g:4 * (g + 1), :]
                                nc.scalar.activation(
                                    dst, tpv, AF.Identity,
                                    bias=dscbt[:, 2 * br + occ:
                                               2 * br + occ + 1],
                                    accum_out=dssum[occ][:, g:g + 1])
                        # GN + ReLU -> fmcat chunks
                        with tc.tile_pool(name="psgn2", bufs=1,
                                          space="PSUM") as psg2:
                            for occ in range(2):
                                nc.vector.tensor_reduce(
                                    scol2[:, occ:occ + 1], dssum[occ][:],
                                    mybir.AxisListType.X, ALU.add)
                                nc.vector.scalar_tensor_tensor(
                                    sqs[:], scr[occ][:], 1.0, scr[occ][:],
                                    ALU.mult, ALU.mult,
                                    accum_out=qcol2[:, occ:occ + 1])
                                if DEBUG and u == 0 and br == 0 and occ == 0:
                                    nc.sync.dma_start(io["dbg_dsc"][:],
                                                      scr[0][:])
                                S, Bc = gn_scale_bias(
                                    psg2, 128, scol2[:, occ:occ + 1],
                                    qcol2[:, occ:occ + 1], 4 * 1024,
                                    g4t_[:], g4tT[:],
                                    dscgt[:, 2 * br + occ:2 * br + occ + 1],
                                    dscbet[:, 2 * br + occ:2 * br + occ + 1],
                                    32, "d")
                                nc.scalar.activation(
                                    img(fmcat, 2 + 2 * br + occ)[:, 1:33,
                                                                 1:33],
                                    scr[occ][:].rearrange(
                                        "p (y x) -> p y x", x=32),
                                    AF.Relu, bias=Bc[:], scale=S[:])
            if DEBUG and u == 0:
                nc.sync.dma_start(io["dbg_fmcat"][:], fmcat[:])

            # ---- P6: c1 conv + GN + ReLU -> fmc[u] ----
            with tc.tile_pool(name="ps6", bufs=1, space="PSUM") as pc, \
                 tc.tile_pool(name="sbw6", bufs=2) as sbw:
                cps = [[pc.tile([128, RG[rg][1] * PW], F32, name=f"c1ps{rg}{oc}", tag=f"c1ps{rg}{oc}")
                        for oc in range(2)] for rg in range(3)]
                for si in range(9):
                    dy, dx = si // 3 - 1, si % 3 - 1
                    delta = PW * dy + dx
                    blk = sbw.tile([128, 1536], F32, name="c1blk", tag="c1blk")
                    nc.sync.dma_start(blk[:], io["c1wT"][:, si * 1536:
                                                         (si + 1) * 1536])
                    for cc in range(6):
                        for rg in range(3):
                            r0, nr = RG[rg]
                            nfree = nr * PW
                            rhs = fmcat[:, FT * cc + SL + r0 * PW + delta:
                                        FT * cc + SL + r0 * PW + delta + nfree]
                            for oc in range(2):
                                nc.tensor.matmul(
                                    cps[rg][oc][:],
                                    blk[:, 256 * cc + 128 * oc:
                                        256 * cc + 128 * (oc + 1)],
                                    rhs,
                                    start=(si == 0 and cc == 0),
                                    stop=(si == 8 and cc == 5))
                for rg in range(3):
                    r0, nr = RG[rg]
                    y0, ny = _interior(rg)
                    for oc in range(2):
                        src = cps[rg][oc][:].rearrange(
                            "p (y x) -> p y x", x=PW)[
                            :, y0 + 1 - r0:y0 + 1 - r0 + ny, 1:33]
                        dst = scr[oc][:].rearrange(
                            "p (y x) -> p y x", x=32)[:, y0:y0 + ny, :]
                        nc.scalar.activation(
                            dst, src, AF.Identity, bias=c1bt[:, oc:oc + 1],
                            accum_out=stats[:, 3 * oc + rg:3 * oc + rg + 1])
                with tc.tile_pool(name="psgn3", bufs=1, space="PSUM") as psg3:
                    for oc in range(2):
                        nc.vector.tensor_reduce(
                            scol2[:, oc:oc + 1], stats[:, 3 * oc:3 * oc + 3],
                            mybir.AxisListType.X, ALU.add)
                        nc.vector.scalar_tensor_tensor(
                            sqs[:], scr[oc][:], 1.0, scr[oc][:], ALU.mult,
                            ALU.mult, accum_out=qcol2[:, oc:oc + 1])
                        S, Bc = gn_scale_bias(
                            psg3, 128, scol2[:, oc:oc + 1],
                            qcol2[:, oc:oc + 1], 4 * 1024, g4t_[:], g4tT[:],
                            c1gt[:, oc:oc + 1], c1bet[:, oc:oc + 1], 32, "e")
                        nc.scalar.activation(
                            fmc[u][:, 1024 * oc:1024 * (oc + 1)], scr[oc][:],
                            AF.Relu, bias=Bc[:], scale=S[:])
            if DEBUG and u == 0:
                nc.sync.dma_start(io["dbg_fmc"][:], fmc[0][:])

        # ============ P7: p2 + exp + attn@v per unit ============
        p2wTt = load("p2wT", [128, 2 * 1024])
        for u in range(2):
            with tc.tile_pool(name="ps7", bufs=2, space="PSUM") as p7, \
                 tc.tile_pool(name="ps7b", bufs=1, space="PSUM") as p7b, \
                 tc.tile_pool(name="sb7", bufs=2) as s7:
                avps = [p7b.tile([33, 512], F32, name=f"avps{ms}", tag=f"avps{ms}")
                        for ms in range(2)]
                for nch in range(8):
                    eps_ = p7.tile([128, 1024], F32, name="p2ps", tag="p2ps")
                    for ms in range(2):
                        for cc in range(2):
                            nc.tensor.matmul(
                                eps_[:, 512 * ms:512 * (ms + 1)],
                                fmc[u][:, 1024 * cc + 128 * nch:
                                       1024 * cc + 128 * (nch + 1)],
                                p2wTt[:, 1024 * cc + 512 * ms:
                                      1024 * cc + 512 * (ms + 1)],
                                start=(cc == 0 and True), stop=False)
                            nc.tensor.matmul(
                                eps_[:, 512 * ms:512 * (ms + 1)],
                                p2wTt[:, 1024 * cc + 128 * nch:
                                      1024 * cc + 128 * (nch + 1)],
                                fmc[u][:, 1024 * cc + 512 * ms:
                                       1024 * cc + 512 * (ms + 1)],
                                start=False, stop=False)
                        nc.tensor.matmul(
                            eps_[:, 512 * ms:512 * (ms + 1)],
                            p2lt[:, 128 * nch:128 * (nch + 1)],
                            p2rt[:, 512 * ms:512 * (ms + 1)],
                            start=False, stop=True)
                    Esb = s7.tile([128, 1024], F32, name="Esb", tag="Esb")
                    nc.scalar.activation(Esb[:], eps_[:], AF.Exp)
                    if DEBUG and u == 0 and nch == 0:
                        nc.sync.dma_start(io["dbg_E"][:], Esb[:])
                    for ms in range(2):
                        nc.tensor.matmul(
                            avps[ms][:],
                            vte[:, 66 * nch + 33 * u:66 * nch + 33 * (u + 1)],
                            Esb[:, 512 * ms:512 * (ms + 1)],
                            start=(nch == 0), stop=(nch == 7))
                # normalize by Z and write gsrc rows
                zt = s7.tile([1, 1024], F32, name="zt", tag="zt")
                for ms in range(2):
                    nc.vector.tensor_copy(zt[:, 512 * ms:512 * (ms + 1)],
                                          avps[ms][32:33, :])
                zb = p7b.tile([32, 1024], F32, name="zb", tag="zb")
                for ms in range(2):
                    nc.tensor.matmul(zb[:, 512 * ms:512 * (ms + 1)],
                                     ones132[:], zt[:, 512 * ms:512 * (ms + 1)],
                                     start=True, stop=True)
                rz = s7.tile([32, 1024], F32, name="rz", tag="rz")
                nc.vector.reciprocal(rz[:], zb[:])
                for ms in range(2):
                    nc.vector.tensor_tensor(
                        gsrc[32 * u:32 * (u + 1), 512 * ms:512 * (ms + 1)],
                        avps[ms][0:32, :], rz[:, 512 * ms:512 * (ms + 1)],
                        ALU.mult)
        if DEBUG:
            nc.sync.dma_start(io["dbg_gsrc"][:], gsrc[:])

        # ============ P8: AllGather + output projection ============
        agin = dramp.tile([64, 1024], F32, name="agin", tag="agin")
        agout = dramp.tile([256, 1024], F32, name="agout", tag="agout")
        nc.sync.dma_start(agin[:], gsrc[:])
        nc.gpsimd.collective_compute(
            "AllGather", ALU.bypass,
            replica_groups=[[0, 1, 2, 3], [4, 5, 6, 7]],
            ins=[agin.opt()], outs=[agout.opt()])
        with tc.tile_pool(name="ps8", bufs=1, space="PSUM") as p8, \
             tc.tile_pool(name="sb8", bufs=1) as s8:
            gat = [s8.tile([128, 1024], F32, name=f"gat{c}", tag=f"gat{c}") for c in range(2)]
            for cc in range(2):
                nc.sync.dma_start(gat[cc][:],
                                  agout[128 * cc:128 * (cc + 1), :])
            wops = p8.tile([64, 1024], F32, name="wops", tag="wops")
            for ms in range(2):
                for cc in range(2):
                    nc.tensor.matmul(
                        wops[:, 512 * ms:512 * (ms + 1)],
                        woTt[:, 64 * cc:64 * (cc + 1)],
                        gat[cc][:, 512 * ms:512 * (ms + 1)],
                        start=(cc == 0), stop=(cc == 1))
            osb = s8.tile([64, 1024], F32, name="osb", tag="osb")
            nc.scalar.activation(osb[:], wops[:], AF.Identity, bias=bo64t[:])
            nc.sync.dma_start(io["ypart"][:], osb[:])


# ======================= host side =======================

def _np(a):
    return np.ascontiguousarray(np.asarray(a, dtype=np.float32))


# 41-row offsets layout: rows 0-26 = [cx_y, cx_x, cy_y], 27-31 zero pad,
# rows 32-40 = cy_x (32-aligned for the tanh/coord reads)
OROWS = list(range(0, 27)) + [None] * 5 + list(range(27, 36))


def _consts():
    c = {}
    c["ident"] = np.eye(128, dtype=np.float32)
    sel4 = np.zeros((4, 128), np.float32)
    for p in range(128):
        sel4[p // 32, p] = 1.0
    c["sel4"] = sel4
    tri = np.zeros((10, 9), np.float32)
    for k in range(9):
        if k > CENTER:
            tri[CENTER + 1:k + 1, k] = 1.0
        elif k < CENTER:
            tri[k:CENTER, k] = 1.0
    tri[9, :] = 1.0
    c["tri"] = tri
    c["niota"] = -(np.arange(128) % 32).astype(np.float32).reshape(128, 1)
    g4 = np.zeros((128, 32), np.float32)
    for p in range(128):
        g4[p, p // 4] = 1.0
    c["g4"] = g4
    c["g4t"] = g4.T.copy()
    gp2 = np.zeros((41, 18), np.float32)
    for row, oc in enumerate(OROWS):
        if oc is not None:
            gp2[row, oc // 2] = 1.0
    c["gp2"] = gp2
    c["gp2t"] = gp2.T.copy()
    for nm, vals in (("gridy", np.arange(32, dtype=np.float32)[:, None] *
                      np.ones((1, 32), np.float32)),
                     ("gridx", np.ones((32, 1), np.float32) *
                      np.arange(32, dtype=np.float32)[None, :])):
        arr = np.zeros((1, FT), np.float32)
        img = np.zeros((PW, PW), np.float32)
        img[1:33, 1:33] = vals
        arr[0, SL:SL + PL] = img.reshape(-1)
        c[nm] = arr
    return c


def _pack_conv(wmat, n_cc, n_o):
    """wmat list of 9 arrays [cin, n_o] -> [128, 9*n_cc*n_o]."""
    out = np.zeros((128, 9 * n_cc * n_o), np.float32)
    for si in range(9):
        for cc in range(n_cc):
            out[:, (si * n_cc + cc) * n_o:(si * n_cc + cc + 1) * n_o] = \
                wmat[si][128 * cc:128 * (cc + 1), :]
    return out


def kernel(**inputs):
    nc = build_program()
    x = _np(inputs["x"])
    consts = _consts()

    base = dict(consts)
    base["p1wT"] = np.zeros((128, 8 * 256), np.float32)
    p1T = _np(inputs["p1_w"]).T        # [1024, 256]
    for mc in range(8):
        for cc in range(2):
            base["p1wT"][:, 256 * mc + 128 * cc:256 * mc + 128 * (cc + 1)] = \
                p1T[128 * mc:128 * (mc + 1), 128 * cc:128 * (cc + 1)]
    base["p1b"] = _np(inputs["p1_b"]).reshape(2, 128).T.copy()
    p2T = _np(inputs["p2_w"]).T        # [256, 1024]
    base["p2wT"] = np.concatenate([p2T[0:128, :], p2T[128:256, :]], axis=1)
    p2b = _np(inputs["p2_b"])
    base["p2l"] = np.stack([np.ones(1024, np.float32), p2b])
    base["p2r"] = np.stack([p2b, np.ones(1024, np.float32)])

    c0_w = _np(inputs["c0_w"])
    base["c0wT"] = _pack_conv(
        [c0_w[:, :, si // 3, si % 3].T.copy() for si in range(9)], 2, 256)
    woff0 = np.concatenate([_np(inputs["cx_off_w"]),
                            _np(inputs["cy_off_w"])], axis=0)
    woff = np.zeros((41, 256, 3, 3), np.float32)
    for row, oc in enumerate(OROWS):
        if oc is not None:
            woff[row] = woff0[oc]
    base["offwT"] = _pack_conv(
        [woff[:, :, si // 3, si % 3].T.copy() for si in range(9)], 2, 41)
    dsc = np.zeros((128, 2 * 9 * 2 * 256), np.float32)
    cxw = _np(inputs["cx_dsc_w"])      # [256,256,9,1]
    cyw = _np(inputs["cy_dsc_w"])      # [256,256,1,9]
    for br, wsrc in ((0, cxw), (1, cyw)):
        for k in range(9):
            wm = (wsrc[:, :, k, 0] if br == 0 else wsrc[:, :, 0, k]).T
            for cc in range(2):
                col = ((br * 9 + k) * 2 + cc) * 256
                dsc[:, col:col + 256] = wm[128 * cc:128 * (cc + 1), :]
    base["dscwT"] = dsc
    c1_w = _np(inputs["c1_w"])          # [256, 768, 3, 3]
    base["c1wT"] = _pack_conv(
        [c1_w[:, :, si // 3, si % 3].T.copy() for si in range(9)], 6, 256)

    def col2(v):
        return _np(v).reshape(2, 128).T.copy()

    base["c0b"] = col2(inputs["c0_b"]); base["c1b"] = col2(inputs["c1_b"])
    base["c0g"] = col2(inputs["c0_g"]); base["c0be"] = col2(inputs["c0_be"])
    base["c1g"] = col2(inputs["c1_g"]); base["c1be"] = col2(inputs["c1_be"])
    def pad41(v):
        out = np.zeros((41, 1), np.float32)
        for row, oc in enumerate(OROWS):
            if oc is not None:
                out[row, 0] = v[oc]
        return out

    base["offb"] = pad41(np.concatenate([_np(inputs["cx_off_b"]),
                                         _np(inputs["cy_off_b"])]))
    base["offg"] = pad41(np.concatenate([_np(inputs["cx_gno_g"]),
                                         _np(inputs["cy_gno_g"])]))
    base["offbe"] = pad41(np.concatenate([_np(inputs["cx_gno_b"]),
                                          _np(inputs["cy_gno_b"])]))
    dscb = np.zeros((128, 4), np.float32)
    dscg = np.zeros((128, 4), np.float32)
    dscbe = np.zeros((128, 4), np.float32)
    for br, pre in ((0, "cx"), (1, "cy")):
        for oc in range(2):
            dscb[:, 2 * br + oc] = _np(inputs[f"{pre}_dsc_b"])[128 * oc:
                                                              128 * (oc + 1)]
            dscg[:, 2 * br + oc] = _np(inputs[f"{pre}_gn_g"])[128 * oc:
                                                             128 * (oc + 1)]
            dscbe[:, 2 * br + oc] = _np(inputs[f"{pre}_gn_b"])[128 * oc:
                                                              128 * (oc + 1)]
    base["dscb"], base["dscg"], base["dscbe"] = dscb, dscg, dscbe

    wq, wk, wv, wo = (_np(inputs[n]) for n in ("wq", "wk", "wv", "wo"))
    bq, bk, bv, bo = (_np(inputs[n]) for n in ("bq", "bk", "bv", "bo"))

    in_maps = []
    for core in range(N_CORES):
        b = core // 4
        j = core % 4
        rs = slice(64 * j, 64 * (j + 1))
        d = dict(base)
        xb = x[b].reshape(256, 1024)
        d["xf"] = np.concatenate([xb[0:128], xb[128:256]], axis=1).copy()
        wqT = (wq[rs, :] * SCALE).T
        d["wqT"] = np.concatenate([wqT[0:128], wqT[128:256]], axis=1).copy()
        wkT = wk[rs, :].T
        d["wkT"] = np.concatenate([wkT[0:128], wkT[128:256]], axis=1).copy()
        wvT = wv[rs, :].T
        d["wvT"] = np.concatenate([wvT[0:128], wvT[128:256]], axis=1).copy()
        d["bq2"] = (bq[rs] * SCALE).reshape(64, 1).copy()
        d["bk2"] = bk[rs].reshape(64, 1).copy()
        d["bv2"] = np.tile(bv[rs].reshape(1, 64), (128, 1)).copy()
        woT = wo[rs, :].T.copy()       # [256, 64]
        d["woT"] = np.concatenate([woT[0:128, :], woT[128:256, :]], axis=1)
        d["bo64"] = bo[rs].reshape(64, 1).copy()
        in_maps.append(d)

    results = _run_cached(nc, in_maps)
    out = np.zeros((2, 256, 32, 32), np.float32)
    for core in range(N_CORES):
        b, j = core // 4, core % 4
        out[b, 64 * j:64 * (j + 1)] = \
            results[core]["ypart"].reshape(64, 32, 32)
    if DEBUG:
        class _R:
            pass
        r = _R()
        r.results = results
        kernel.last_results = r
    return out


def _get_runner(nc):
    if "runner" in _CACHE:
        return _CACHE["runner"]
    import jax
    from jax.experimental.shard_map import shard_map
    from jax.sharding import Mesh, PartitionSpec
    from concourse import bass2jax as b2j
    b2j.install_neuronx_cc_hook()

    pname = nc.partition_id_tensor.name if nc.partition_id_tensor else None
    in_names, out_names, out_avals = [], [], []
    for alloc in nc.m.functions[0].allocations:
        if not isinstance(alloc, mybir.MemoryLocationSet):
            continue
        name = alloc.memorylocations[0].name
        if alloc.kind == "ExternalInput":
            if name != pname:
                in_names.append(name)
        elif alloc.kind == "ExternalOutput":
            shape = tuple(alloc.tensor_shape)
            dtype = mybir.dt.np(alloc.dtype)
            out_names.append(name)
            out_avals.append(jax.core.ShapedArray(shape, dtype))
    n_params = len(in_names)
    all_names = in_names + out_names
    if pname is not None:
        all_names = all_names + [pname]

    def _body(*args):
        operands = list(args)
        if pname is not None:
            operands.append(b2j.partition_id_tensor())
        outs = b2j._bass_exec_p.bind(
            *operands, out_avals=tuple(out_avals), in_names=tuple(all_names),
            out_names=tuple(out_names), lowering_input_output_aliases=(),
            sim_require_finite=True, sim_require_nnan=True, nc=nc)
        return tuple(outs)

    devices = jax.devices()[:N_CORES]
    mesh = Mesh(np.asarray(devices), ("core",))
    n_out = len(out_names)
    sharded = jax.jit(shard_map(
        _body, mesh=mesh,
        in_specs=(PartitionSpec("core"),) * (n_params + n_out),
        out_specs=(PartitionSpec("core"),) * n_out, check_rep=False))
    _CACHE["runner"] = (sharded, in_names, out_names, out_avals, mesh)
    return _CACHE["runner"]


def _concat_inputs(in_maps, in_names, out_avals):
    concat = [np.concatenate([np.asarray(m[n]) for m in in_maps], axis=0)
              for n in in_names]
    zeros = [np.zeros((N_CORES * a.shape[0],) + a.shape[1:], a.dtype)
             for a in out_avals]
    return concat + zeros


def _run_cached(nc, in_maps):
    sharded, in_names, out_names, out_avals, mesh = _get_runner(nc)
    args = _concat_inputs(in_maps, in_names, out_avals)
    out_arrs = sharded(*args)
    return [
        {n: np.asarray(out_arrs[i]).reshape(N_CORES, *out_avals[i].shape)[c]
         for i, n in enumerate(out_names)}
        for c in range(N_CORES)
    ]





if __name__ == "__main__":
    build_program()
    print("built ok")
